# revision 1
# baseline (speedup 1.0000x reference)
"""DAHHConv (hypergraph conv) Trainium2 Bass kernel, 8-core SPMD.

Math (reference):
    x' = x @ theta                      # [B,N,C]
    xe = (H^T x') / deg_e               # [B,E,C], deg_e = sum_n H
    xn = (H xe) / deg_n                 # [B,N,C], deg_n = sum_e H
    out = xn + bias

Sharding: 8 cores = 4 batches x 2 halves; core c -> batch b=c//2, half
h=c%2. Phase 1 (edge aggregation, contraction over n) shards E: each
core owns e in [1024h, 1024h+1024) with all N rows local (no
reduction). Phase 3 (node aggregation, contraction over e) shards N:
each core owns n in [4096h, 4096h+4096) and needs the full E range —
the m_e halves are exchanged pairwise with a 2-rank AllGather (bf16
payload). H is supplied host-side in BOTH layouts as fp8 (exact for a
0/1 incidence matrix; the PE accepts bf16 lhsT x fp8 rhs): an n-major
slice for phase 1 and a transposed e-major slice for phase 3.

Layout tricks:
  - deg_e / deg_n come free as a ones-column in the stationary operand.
  - hn rows are consumed in a host-permuted order (4 consecutive DRAM
    rows per SBUF partition) so every DMA line is 4KB contiguous; the
    n-sum is order-invariant so only xt's column order must match.
  - phase 3 keeps y^T in [feature, node] layout: deg_n division uses a
    GpSimd partition-broadcast of the reciprocal deg row, the output is
    written transposed (16KB DMA lines) and the host transposes back.
"""

import numpy as np
import ml_dtypes

B, N, E, C = 4, 8192, 2048, 64
NCORES = 8
EH = E // 2          # 1024: e-range per core in phase 1
NH = N // 2          # 4096: n-range per core in phase 3
CA = C + 1           # 65: feature dim augmented with ones/deg column
NCHUNK = N // 128    # 64 n-chunks in phase 1
HNTILES = N // 512   # 16 hn DMA tiles (512 rows each)
ECHUNK = E // 128    # 16 e-chunks in phase 3
NSPAN = 1024         # phase-3 output span (2 PSUM banks at fp32)
BF16 = ml_dtypes.bfloat16
FP8 = ml_dtypes.float8_e4m3

_cache = {}


def _split_waits_json(raw: bytes) -> bytes:
    """BIR post-pass: this walrus/ISA build allows only ONE sync wait per
    instruction, but the Tile scheduler attaches several. Hoist all but
    the last wait of each instruction onto standalone EventSemaphore
    instructions inserted just before it on the same engine (waits are
    pure preconditions, so running them earlier on the same engine
    stream is equivalent)."""
    import json

    m = json.loads(raw)
    ctr = 0
    for f in m["functions"]:
        for blk in f["blocks"]:
            new = []
            for inst in blk["instructions"]:
                si = inst.get("sync_info")
                waits = (si or {}).get("on_wait") or []
                if len(waits) > 1:
                    for w in waits[:-1]:
                        ctr += 1
                        new.append(
                            {
                                "debug": inst.get("debug", 0),
                                "engine": inst["engine"],
                                "ins": [],
                                "name": f"{inst['name']}-xw{ctr}",
                                "opcode": "EventSemaphore",
                                "outs": [],
                                "sync_info": {"on_update": [], "on_wait": [w]},
                            }
                        )
                    si["on_wait"] = [waits[-1]]
                new.append(inst)
            blk["instructions"] = new
    return json.dumps(m).encode()


def build_bass():
    import concourse.bass as bass
    import concourse.mybir as mybir
    from concourse.tile import TileContext
    from concourse import masks

    dt = mybir.dt
    nc = bass.Bass()

    hn = nc.declare_dram_parameter("hn", [N, EH], dt.float8e4, isOutput=False)
    ht = nc.declare_dram_parameter("ht", [E, NH], dt.float8e4, isOutput=False)
    xt = nc.declare_dram_parameter("xt", [C, N], dt.bfloat16, isOutput=False)
    th = nc.declare_dram_parameter("th", [C, C], dt.bfloat16, isOutput=False)
    out = nc.declare_dram_parameter("out", [C, NH], dt.float32, isOutput=True)

    # collective bounce buffers (DRAM; SBUF collectives are banned)
    cc_in = nc.dram_tensor("cc_in", [CA, EH], dt.bfloat16)
    cc_out = nc.dram_tensor("cc_out", [2 * CA, EH], dt.bfloat16)
    # per-span deg_n staging rows (DRAM hop: repartition + broadcast DMA)
    ddram = nc.dram_tensor("ddram", [NH // NSPAN, NSPAN], dt.float32)
    rdram = nc.dram_tensor("rdram", [NH // NSPAN, NSPAN], dt.float32)

    with TileContext(nc) as tc:
        with (
            tc.tile_pool(name="const", bufs=1) as const,
            tc.tile_pool(name="persist", bufs=1) as persist,
            tc.tile_pool(name="hn_pool", bufs=5) as hn_pool,
            tc.tile_pool(name="ht_pool", bufs=1) as ht_pool,
            tc.tile_pool(name="small", bufs=2) as small,
        ):
            ident = const.tile([128, 128], dt.float32)
            masks.make_identity(nc, ident[:])
            ones_f32 = const.tile([1, C], dt.float32)
            nc.vector.memset(ones_f32[:], 1.0)
            th_sb = const.tile([C, C], dt.bfloat16)
            nc.sync.dma_start(th_sb[:], th[:])
            xt_sb = persist.tile([C, N], dt.bfloat16)
            for q in range(4):
                nc.sync.dma_start(
                    xt_sb[:, 2048 * q : 2048 * (q + 1)],
                    xt[:, 2048 * q : 2048 * (q + 1)],
                )
            # all 16 ht tiles stay resident; 10 stream inside the phase-1
            # hn stream, 6 during the collective window
            ht_tiles = [
                ht_pool.tile([128, NH], dt.float8e4, tag=f"ht{k}", name=f"ht{k}")
                for k in range(ECHUNK)
            ]

            # x'_aug chunks: chunk j at cols [65j, 65j+65); col 65j+64 = 1
            xp_sb = persist.tile([128, CA * NCHUNK], dt.bfloat16)
            xp_v = xp_sb[:].rearrange("p (c w) -> p c w", w=CA)
            nc.vector.memset(xp_v[:, :, C : C + 1], 1.0)

            # ---- phase 0: x' = x @ theta (theta stationary per chunk) ----
            with tc.tile_pool(name="ps0", bufs=2, space="PSUM") as ps0:
                for blk in range(NCHUNK // 8):
                    ps_xp = ps0.tile([128, 8 * C], dt.float32)
                    for jj in range(8):
                        j = 8 * blk + jj
                        nc.tensor.matmul(
                            ps_xp[:, C * jj : C * (jj + 1)],
                            xt_sb[:, 128 * j : 128 * (j + 1)],
                            th_sb[:],
                        )
                    src = ps_xp[:].rearrange("p (c w) -> p c w", w=C)
                    dst = xp_v[:, 8 * blk : 8 * (blk + 1), 0:C]
                    nc.vector.tensor_copy(dst, src)

            # ---- phase 1: m_e^T[65,1024] = x'_aug^T @ H_n  (accum) ----
            # hn tile t covers DRAM rows [512t, 512t+512): partition p
            # holds rows 512t+4p..512t+4p+3 (4KB contiguous lines); the
            # matching x' chunks are j = 4t..4t+3 (xt is host-permuted).
            with tc.tile_pool(name="ps1", bufs=1, space="PSUM") as ps1:
                ps_me = ps1.tile([CA, EH], dt.float32)
                for t in range(HNTILES):
                    hn_t = hn_pool.tile([128, 4 * EH], dt.float8e4)
                    src = hn[512 * t : 512 * (t + 1), :].rearrange(
                        "(p four) e -> p (four e)", four=4
                    )
                    nc.sync.dma_start(hn_t[:], src)
                    if 1 <= t <= 10:
                        k = t - 1
                        nc.sync.dma_start(
                            ht_tiles[k][:], ht[128 * k : 128 * (k + 1), :]
                        )
                    for q in range(4):
                        j = 4 * t + q
                        for half in range(2):
                            nc.tensor.matmul(
                                ps_me[:, 512 * half : 512 * (half + 1)],
                                xp_sb[:, CA * j : CA * (j + 1)],
                                hn_t[:, 1024 * q + 512 * half : 1024 * q + 512 * (half + 1)],
                                start=(t == 0 and q == 0),
                                stop=(t == HNTILES - 1 and q == 3),
                            )
                me_sb = small.tile([CA, EH], dt.bfloat16)
                nc.vector.tensor_copy(me_sb[:], ps_me[:])
                nc.sync.dma_start(cc_in[:], me_sb[:])

            # remaining ht tiles stream while the collective runs
            for k in range(10, ECHUNK):
                nc.sync.dma_start(ht_tiles[k][:], ht[128 * k : 128 * (k + 1), :])

            # ---- exchange: 2-rank AllGather within each batch pair ----
            nc.gpsimd.collective_compute(
                "AllGather",
                mybir.AluOpType.bypass,
                replica_groups=[[0, 1], [2, 3], [4, 5], [6, 7]],
                ins=[cc_in[:]],
                outs=[cc_out[:]],
            )

            # ---- phase 2: xe_aug[e,65] = (m_e/deg_e, 1), e on partitions ----
            xe_sb = persist.tile([128, CA * ECHUNK], dt.bfloat16)
            xe_v = xe_sb[:].rearrange("p (c w) -> p c w", w=CA)
            nc.vector.memset(xe_v[:, :, C : C + 1], 1.0)
            with tc.tile_pool(name="ps2", bufs=2, space="PSUM") as ps2:
                for r in range(2):
                    mr16 = small.tile([CA, EH], dt.bfloat16, tag="mr16")
                    nc.sync.dma_start(mr16[:], cc_out[CA * r : CA * (r + 1), :])
                    mr = small.tile([CA, EH], dt.float32, tag="mr")
                    nc.vector.tensor_copy(mr[:], mr16[:])
                    for t in range(EH // 128):
                        k = (EH // 128) * r + t
                        ps_tr = ps2.tile([128, CA], dt.float32)
                        nc.tensor.transpose(
                            ps_tr[:], mr[:, 128 * t : 128 * (t + 1)], ident[0:CA, 0:CA]
                        )
                        rec = small.tile([128, 1], dt.float32, tag="rec")
                        nc.vector.reciprocal(rec[:], ps_tr[:, C : C + 1])
                        nc.vector.tensor_scalar_mul(
                            xe_v[:, k, 0:C], ps_tr[:, 0:C], rec[:]
                        )

            # ---- phase 3: y^T[65,span] = xe_aug^T @ H_e^T; out = y/deg_n ----
            # software-pipelined: span s+1's matmuls are emitted before
            # span s's post-processing so the PE stream stays dense.
            with tc.tile_pool(name="ps3", bufs=2, space="PSUM") as ps3:
                nspans = NH // NSPAN
                span_ps = {}

                def span_mms(s):
                    ps_y = ps3.tile(
                        [CA, NSPAN], dt.float32, tag="ps_y", name=f"ps_y{s}"
                    )
                    span_ps[s] = ps_y
                    for k in range(ECHUNK):
                        for half in range(2):
                            col = NSPAN * s + 512 * half
                            nc.tensor.matmul(
                                ps_y[:, 512 * half : 512 * (half + 1)],
                                xe_sb[:, CA * k : CA * (k + 1)],
                                ht_tiles[k][:, col : col + 512],
                                start=(k == 0),
                                stop=(k == ECHUNK - 1),
                            )

                def span_post(s):
                    ps_y = span_ps[s]
                    # deg_n row -> [128, 8] so the reciprocal runs on all
                    # 128 DVE lanes (a [1, NSPAN] op would serialize on one)
                    drow = small.tile([1, NSPAN], dt.float32, tag="drow")
                    nc.vector.tensor_copy(drow[:], ps_y[C : C + 1, :])
                    nc.sync.dma_start(ddram[s : s + 1, :], drow[:])
                    dcol = small.tile([128, NSPAN // 128], dt.float32, tag="dcol")
                    nc.sync.dma_start(
                        dcol[:],
                        ddram[s : s + 1, :].rearrange(
                            "one (p f) -> (one p) f", p=128
                        ),
                    )
                    rcol = small.tile([128, NSPAN // 128], dt.float32, tag="rcol")
                    nc.vector.reciprocal(rcol[:], dcol[:])
                    nc.sync.dma_start(
                        rdram[s : s + 1, :].rearrange("one (p f) -> (one p) f", p=128),
                        rcol[:],
                    )
                    # stride-0 partition broadcast of the staged row
                    rrep = small.tile([C, NSPAN], dt.float32, tag="rrep")
                    bcast_src = bass.AP(
                        tensor=rdram,
                        offset=s * NSPAN,
                        ap=[[0, C], [1, NSPAN]],
                    )
                    nc.sync.dma_start(rrep[:], bcast_src)
                    o_sb = small.tile([C, NSPAN], dt.float32, tag="o_sb")
                    nc.vector.tensor_tensor(
                        o_sb[:], ps_y[0:C, :], rrep[:], mybir.AluOpType.mult
                    )
                    nc.sync.dma_start(
                        out[:, NSPAN * s : NSPAN * (s + 1)], o_sb[:]
                    )

                span_mms(0)
                span_mms(1)
                span_post(0)
                span_mms(2)
                span_post(1)
                span_mms(3)
                span_post(2)
                span_post(3)

    orig_to_json = nc.to_json_bytes
    nc.to_json_bytes = lambda: _split_waits_json(orig_to_json())
    return nc


def _fp8_exact(a):
    # H is 0/1: 1.0 is exactly 0x38 in float8_e4m3.
    return (np.where(a != 0, 0x38, 0)).astype(np.uint8).view(FP8)


def _prepare_in_maps(x, H, theta):
    x = np.ascontiguousarray(x, dtype=np.float32)
    H = np.ascontiguousarray(H, dtype=np.float32)
    th16 = np.ascontiguousarray(theta, dtype=np.float32).astype(BF16)
    in_maps = []
    for c in range(NCORES):
        b, h = divmod(c, 2)
        hn = _fp8_exact(np.ascontiguousarray(H[b, :, EH * h : EH * (h + 1)]))
        ht = _fp8_exact(np.ascontiguousarray(H[b, NH * h : NH * (h + 1), :].T))
        # phase-1 consumes n in blocks of 512 as [128 partitions x 4 rows]:
        # chunk j = 4t+q, partition p <-> DRAM row 512t+4p+q. Permute xt's
        # columns to match (the n-contraction is order-invariant).
        xtb = x[b].T.reshape(C, HNTILES, 128, 4)
        xtp = np.ascontiguousarray(
            xtb.transpose(0, 1, 3, 2).reshape(C, N)
        ).astype(BF16)
        in_maps.append({"hn": hn, "ht": ht, "xt": xtp, "th": th16})
    return in_maps


def _assemble(results, bias):
    out = np.empty((B, N, C), dtype=np.float32)
    for c in range(NCORES):
        b, h = divmod(c, 2)
        out[b, NH * h : NH * (h + 1), :] = results[c]["out"].T
    out += np.asarray(bias, dtype=np.float32)[None, None, :]
    return out


def get_nc():
    if "nc" not in _cache:
        _cache["nc"] = build_bass()
    return _cache["nc"]


def kernel(x, H, theta, bias):
    from concourse.bass_utils import run_bass_kernel_spmd

    nc = get_nc()
    in_maps = _prepare_in_maps(x, H, theta)
    res = run_bass_kernel_spmd(nc, in_maps, list(range(NCORES)))
    return _assemble(res.results, bias)

